# revision 21
# baseline (speedup 1.0000x reference)
"""Trainium2 Bass kernel for nn_LowerBlock (binarized 1x1 conv block).

Computes, per NCHW f32 input x[64,512,28,28]:
    a   = sign(x + rsign_bias)                        (RSign, forward=sign)
    y   = einsum('bchw,oc->bohw', a, sign(W)*mean|W|) (scaled-sign 1x1 conv)
    bn  = gamma*(y-mean)*rsqrt(var+eps) + beta        (BatchNorm2d inference)
    s   = bn + x                                      (residual)
    out = prelu(s - pr_shift; pr_slope) + pr_bias     (RPReLU)

Strategy: data-parallel over batch across 8 NeuronCores (8 samples/core).
All per-channel affine algebra (conv scale, BN, rsign/pr shifts, weight
row-sums) is folded on the host into per-channel vectors so the on-chip work
per [128ch x 784px] tile is just:
    a' = (x >= -rsign_bias)            DVE tensor_scalar is_ge, {0,1} bf16, 2x
    S' = signW.T @ a'                  8 bf16 matmuls accumulating in PSUM
    w  = A2*S' + x                     DVE scalar_tensor_tensor from PSUM
    v  = prelu(w + B0p; pr_slope)      ACT Prelu (per-partition bias+alpha)
    y  = v + pr_bias                   ACT Identity (per-partition bias)

The matmul is exact: a' in {0,1} (bf16 exact), weights sign(W).T in {-1,+1}
(bf16 exact), so PSUM accumulates exact small integers; the true signed conv
equals 2*S' - rowsum(sign W), folded into A2 = 2*A and B0p = B0 - A*rowsum.
End-to-end relative error vs the f32 reference is ~2.5e-7.

HBM traffic is the roofline: 25.7 MB/core at ~360 GB/s -> ~72 us floor;
measured ~90-95 us/iteration on HW (TimelineSim: 83 us).
"""
import numpy as np
import ml_dtypes

B, C, H, W_ = 64, 512, 28, 28
HW = H * W_          # 784
NCORES = 8
BPC = B // NCORES    # samples per core
NCH = C // 128       # 4 channel chunks
BN_EPS = 1e-5

_cached = {}


def _build_nc(repeat=0, out_q="sync"):
    """repeat>0 wraps the whole per-core computation in a For_i executed
    `repeat` times — used only by the timing harness (slope method)."""
    import contextlib

    import concourse.bacc as bacc
    import concourse.tile as tile
    from concourse import mybir

    AF = mybir.ActivationFunctionType
    dt = mybir.dt
    Alu = mybir.AluOpType

    nc = bacc.Bacc("TRN2", target_bir_lowering=False, debug=False,
                   num_devices=NCORES)
    x_d = nc.dram_tensor("x", [BPC, NCH, 128, HW], dt.float32,
                         kind="ExternalInput")
    wt_d = nc.dram_tensor("wt", [NCH, 128, C], dt.bfloat16,
                          kind="ExternalInput")
    par_d = nc.dram_tensor("par", [NCH, 128, 5], dt.float32,
                           kind="ExternalInput")
    y_d = nc.dram_tensor("y", [BPC, NCH, 128, HW], dt.float32,
                         kind="ExternalOutput")

    with tile.TileContext(nc) as tc:
        with (
            tc.tile_pool(name="singles", bufs=1) as singles,
            tc.tile_pool(name="xp", bufs=6) as xp,
            tc.tile_pool(name="ap", bufs=4) as apool,
            tc.tile_pool(name="tp", bufs=4) as tp,
            tc.tile_pool(name="vp", bufs=4) as vp,
            tc.tile_pool(name="op", bufs=8) as op,
            tc.tile_pool(name="pp", bufs=4, space="PSUM") as pp,
        ):
            wt_sb = singles.tile([128, NCH, C], dt.bfloat16)
            nc.sync.dma_start(out=wt_sb, in_=wt_d[:].rearrange("c p o -> p c o"))
            par_sb = singles.tile([128, NCH, 5], dt.float32)
            nc.sync.dma_start(out=par_sb, in_=par_d[:].rearrange("c p j -> p c j"))

            loop = (tc.For_i(0, repeat, 1,
                             hint_engines=(mybir.EngineType.PE,
                                           mybir.EngineType.DVE,
                                           mybir.EngineType.Activation,
                                           mybir.EngineType.SP))
                    if repeat > 0 else contextlib.nullcontext())
            with loop:
                _emit_body(nc, tc, mybir, AF, dt, Alu,
                           x_d, y_d, wt_sb, par_sb, xp, apool, tp, vp, op, pp,
                           out_q=out_q)

    nc.compile()
    return nc


def _emit_body(nc, tc, mybir, AF, dt, Alu, x_d, y_d, wt_sb, par_sb,
               xp, apool, tp, vp, op, pp, out_q="sync"):
    if True:
            for b in range(BPC):
                x_sb = xp.tile([128, NCH, HW], dt.float32)
                a_sb = apool.tile([128, NCH, HW], dt.bfloat16)
                for c2 in range(2):
                    nc.sync.dma_start(
                        out=x_sb[:, 2 * c2:2 * c2 + 2, :],
                        in_=x_d[b, 2 * c2:2 * c2 + 2].rearrange("c p n -> p c n"))
                for c in range(NCH):
                    nc.vector.tensor_scalar(
                        out=a_sb[:, c, :], in0=x_sb[:, c, :],
                        scalar1=par_sb[:, c, 0:1], scalar2=None, op0=Alu.is_ge)

                for o in range(NCH):
                    if o % 2 == 0:
                        o_sb = op.tile([128, 2, HW], dt.float32)
                    ps = pp.tile([128, HW], dt.float32)
                    for n0, n1 in ((0, 512), (512, HW)):
                        for c in range(NCH):
                            nc.tensor.matmul(
                                ps[:, n0:n1],
                                wt_sb[:, c, o * 128:(o + 1) * 128],
                                a_sb[:, c, n0:n1],
                                start=(c == 0), stop=(c == NCH - 1))
                    w = tp.tile([128, HW], dt.float32)
                    nc.vector.scalar_tensor_tensor(
                        out=w, in0=ps, scalar=par_sb[:, o, 1:2],
                        in1=x_sb[:, o, :], op0=Alu.mult, op1=Alu.add)
                    v = vp.tile([128, HW], dt.float32)
                    nc.scalar.activation(out=v, in_=w, func=AF.Prelu,
                                         bias=par_sb[:, o, 2:3],
                                         alpha=par_sb[:, o, 3:4])
                    nc.scalar.activation(out=o_sb[:, o % 2, :], in_=v,
                                         func=AF.Identity,
                                         bias=par_sb[:, o, 4:5])
                    if o % 2 == 1:
                        eng = nc.sync if out_q == "sync" else nc.scalar
                        eng.dma_start(
                            out=y_d[b, o - 1:o + 1].rearrange("c p n -> p c n"),
                            in_=o_sb)


def _prepare_consts(rsign_bias, W, bn_gamma, bn_beta, bn_mean, bn_var,
                    pr_slope, pr_shift, pr_bias):
    W64 = W.astype(np.float64)
    scale = np.abs(W64).mean(axis=1)
    R = np.sign(W64).sum(axis=1)
    g = bn_gamma.astype(np.float64) / np.sqrt(bn_var.astype(np.float64) + BN_EPS)
    A = g * scale
    B0 = bn_beta.astype(np.float64) - g * bn_mean.astype(np.float64) \
        - pr_shift.astype(np.float64)
    par = np.stack([
        -rsign_bias.astype(np.float64),
        2.0 * A,
        B0 - A * R,
        pr_slope.astype(np.float64),
        pr_bias.astype(np.float64),
    ], axis=-1).astype(np.float32)          # [512, 5]
    par = np.ascontiguousarray(par.reshape(NCH, 128, 5))
    wt = np.ascontiguousarray(
        np.sign(W.astype(np.float32)).T).astype(ml_dtypes.bfloat16)
    wt = np.ascontiguousarray(wt.reshape(NCH, 128, C))
    return wt, par


def _run(inputs, trace=False):
    from concourse.bass_utils import run_bass_kernel_spmd

    if "nc" not in _cached:
        _cached["nc"] = _build_nc()
    nc = _cached["nc"]

    x = np.asarray(inputs["x"], dtype=np.float32)
    wt, par = _prepare_consts(
        np.asarray(inputs["rsign_bias"], np.float32),
        np.asarray(inputs["W"], np.float32),
        np.asarray(inputs["bn_gamma"], np.float32),
        np.asarray(inputs["bn_beta"], np.float32),
        np.asarray(inputs["bn_mean"], np.float32),
        np.asarray(inputs["bn_var"], np.float32),
        np.asarray(inputs["pr_slope"], np.float32),
        np.asarray(inputs["pr_shift"], np.float32),
        np.asarray(inputs["pr_bias"], np.float32),
    )

    xs = np.ascontiguousarray(x.reshape(NCORES, BPC, NCH, 128, HW))
    in_maps = [{"x": xs[i], "wt": wt, "par": par} for i in range(NCORES)]
    res = run_bass_kernel_spmd(nc, in_maps, core_ids=list(range(NCORES)),
                               trace=trace)
    outs = [r["y"].reshape(BPC, C, H, W_) for r in res.results]
    return np.concatenate(outs, axis=0), res


def kernel(**inputs) -> np.ndarray:
    out, _ = _run(inputs, trace=False)
    return out
